# revision 23
# baseline (speedup 1.0000x reference)
"""Trainium2 Bass kernel for nn_Aligner (sparse_attention).

Per batch b (B=16, S=1024, D=1024):
    scores   = h_src[b] @ h_tgt[b].T
    scores_s = scores + add_tgt[None,:]   # tgt-special-token cols masked
    scores_t = scores + add_src[:,None]   # src-special-token rows masked
    align      = (softmax_row(scores_s) > 1e-3) & (softmax_col(scores_t) > 1e-3)
    p          = softmax_row(scores_s / sqrt(len_tgt))
    q          = softmax_col(scores_t / sqrt(len_src))
    align_prob = 2 p q / (p + q + 1e-9)
Returns (align[:,None] bool, align_prob[:,None] float32).

Distribution: pure data parallel, 2 batches per core on 8 NeuronCores.

Device algorithm (all work stays in [s,t] layout; no [S,S] transposes):
  sweep1  PE: scores via fp16x3 split-product matmuls (hi/lo half
          decomposition of the fp32 operands: hi*hi + hi*lo + lo*hi, each
          product exact in the fp32 PSUM accumulate; max error ~1.4e-5,
          validated to flip zero threshold booleans). DVE folds in the
          tgt mask and takes the row max; ACT exp passes (with fused
          row-sum accumulators) produce row sums s1 (untempered) and s2
          (tempered), plus tempered column-sum operands whose column sums
          d2, d' are taken by PE ones-matmuls.
  The untempered column softmax is stabilized per column by a tempered
  log-sum-exp surrogate colmax c'_j = 8 ln(sum_i exp(x/8)) which lies in
  [colmax_j, colmax_j + 55.4] (clamped at -300 for fully-masked cols).
  sweep2  s'_j = sum_i exp(x + add_src - c') via DVE subtract + ACT exp +
          fp32 PE ones-matmul (fp32 because s' feeds a log-domain
          threshold with ~4e-5 budget; reduced-precision sums are not
          safe). align_prob is also produced here:
          2pq/(p+q) = p*(1+tanh(z/2)) with z = ln q - ln p an affine
          function of the scores, so the whole harmonic mean costs one
          DVE op, one ACT tanh, one ACT exp, one DVE combine per block
          (tanh lives in the same ACT table set as exp - no table-set
          reload thrash).
  sweep3  both alignment tests are done in log space:
            bit1:  x > m_i + ln(tau) + ln(s1_i)        (per-row scalar)
            bit2:  x + add_src > ln(tau s'_j) + c'_j   (per-col vector)
          fused into two DVE scalar_tensor_tensor ops producing uint8.
"""

import numpy as np

B, S, D = 16, 1024, 1024
NCORES = 8
BPC = B // NCORES
NBLK = S // 128
PAD_ID, CLS_ID, SEP_ID = 0, 101, 102
TAU = 1e-3
LNTAU = float(np.log(np.float32(TAU)))
MM_MODE = "f16x3"        # "f16x3" | "f32"
PREFETCH = 4             # next batch matmul blocks pre-emitted to the PE

_CACHE = {}


def _build_bass():
    from contextlib import ExitStack
    import concourse.bass as bass
    import concourse.tile as tile
    import concourse.mybir as mybir
    from concourse import bacc
    from concourse.mybir import AluOpType as op, ActivationFunctionType as act

    f32 = mybir.dt.float32
    f16 = mybir.dt.float16
    bf16 = mybir.dt.bfloat16
    u8 = mybir.dt.uint8

    nc = bacc.Bacc(None, target_bir_lowering=False)

    if MM_MODE == "f16x3":
        shi = nc.declare_dram_parameter("shi", [BPC, NBLK, 128, 8, 128], f16, isOutput=False)
        slo = nc.declare_dram_parameter("slo", [BPC, NBLK, 128, 8, 128], f16, isOutput=False)
        thi = nc.declare_dram_parameter("thi", [BPC, 128, 8, S], f16, isOutput=False)
        tlo = nc.declare_dram_parameter("tlo", [BPC, 128, 8, S], f16, isOutput=False)
    else:
        sT = nc.declare_dram_parameter("sT", [BPC, NBLK, 128, 8, 128], f32, isOutput=False)
        tT = nc.declare_dram_parameter("tT", [BPC, 128, 8, S], f32, isOutput=False)
    vaddt = nc.declare_dram_parameter("vaddt", [BPC, S], f32, isOutput=False)
    vsrcR = nc.declare_dram_parameter("vsrcR", [BPC, 128, NBLK], f32, isOutput=False)
    vsrcT = nc.declare_dram_parameter("vsrcT", [BPC, 128, NBLK], f32, isOutput=False)
    vsrc8 = nc.declare_dram_parameter("vsrc8", [BPC, 128, NBLK], f32, isOutput=False)
    vscal = nc.declare_dram_parameter("vscal", [BPC, 128, 3], f32, isOutput=False)
    vsrcT2 = nc.declare_dram_parameter("vsrcT2", [BPC, 128, NBLK], f32, isOutput=False)
    oal = nc.declare_dram_parameter("oal", [BPC, S, S], u8, isOutput=True)
    opr = nc.declare_dram_parameter("opr", [BPC, S, S], f32, isOutput=True)

    with tile.TileContext(nc) as tc, ExitStack() as es:
        io = es.enter_context(tc.tile_pool(name="io", bufs=1))
        srcp = es.enter_context(tc.tile_pool(name="srcp", bufs=4))
        big = es.enter_context(tc.tile_pool(name="big", bufs=1))
        sc = es.enter_context(tc.tile_pool(name="sc", bufs=4))
        sc2 = es.enter_context(tc.tile_pool(name="sc2", bufs=3))
        bc = es.enter_context(tc.tile_pool(name="bc", bufs=1))
        accp = es.enter_context(tc.tile_pool(name="accp", bufs=1))
        vecp = es.enter_context(tc.tile_pool(name="vecp", bufs=1))
        const = es.enter_context(tc.tile_pool(name="const", bufs=1))
        drp = es.enter_context(tc.tile_pool(name="drp", bufs=2, space="DRAM"))
        mmp = es.enter_context(tc.tile_pool(name="mmp", bufs=2, space="PSUM"))
        csp = es.enter_context(tc.tile_pool(name="csp", bufs=1, space="PSUM"))

        onesb = const.tile([128, 1], bf16, tag="onesb", name="onesb")
        nc.vector.memset(onesb, 1.0)
        onesh = const.tile([128, 1], f16, tag="onesh", name="onesh")
        nc.vector.memset(onesh, 1.0)
        onesS = const.tile([128, 1], f32, tag="onesS", name="onesS")
        nc.vector.memset(onesS, TAU)
        cEPS12 = const.tile([128, 1], f32, tag="cEPS12", name="cEPS12")
        nc.vector.memset(cEPS12, 1e-12)
        cEPS38 = const.tile([128, 1], f32, tag="cEPS38", name="cEPS38")
        nc.vector.memset(cEPS38, 1e-38)

        cache = {}

        def emit_loads(b):
            key = ("L", b)
            if key in cache:
                return cache[key]
            L = {}
            if MM_MODE == "f16x3":
                L["t_hi"] = io.tile([128, 8, S], f16, tag="thi", name="thi")
                L["t_lo"] = io.tile([128, 8, S], f16, tag="tlo", name="tlo")
                nc.gpsimd.dma_start(out=L["t_hi"], in_=thi[b])
            else:
                L["t_f"] = io.tile([128, 8, S], f32, tag="tT", name="tT")
                nc.gpsimd.dma_start(out=L["t_f"], in_=tT[b])
            for nm, src_t, w in (("vsrcR", vsrcR, NBLK), ("vsrcT", vsrcT, NBLK),
                                 ("vsrc8", vsrc8, NBLK), ("vscal", vscal, 3),
                                 ("vsrcT2", vsrcT2, NBLK)):
                L[nm] = io.tile([128, w], f32, tag=nm, name=nm)
                nc.gpsimd.dma_start(out=L[nm], in_=src_t[b])
            L["addt_vec"] = vecp.tile([1, S], f32, tag="addt_vec", name="addt_vec")
            nc.gpsimd.dma_start(out=L["addt_vec"], in_=vaddt[b:b + 1, :])
            L["addtB"] = bc.tile([128, S], f32, tag="addtB", name="addtB")
            vb = vaddt[b]
            nc.gpsimd.dma_start(
                out=L["addtB"],
                in_=bass.AP(tensor=vb.tensor, offset=vb.offset,
                            ap=[[0, 128]] + list(vb.ap)))
            cache[key] = L
            return L

        def emit_fwd(b, k):
            """Scores matmuls for one 128-row block -> PSUM tile."""
            key = ("F", b, k)
            if key in cache:
                return cache[key]
            L = emit_loads(b)
            ps = mmp.tile([128, S], f32, tag="mm", name="mm")
            if MM_MODE == "f16x3":
                s_hi = srcp.tile([128, 8, 128], f16, tag="shi", name="shi")
                s_lo = srcp.tile([128, 8, 128], f16, tag="slo", name="slo")
                nc.gpsimd.dma_start(out=s_hi, in_=shi[b, k])
                nc.gpsimd.dma_start(out=s_lo, in_=slo[b, k])
                if k == 0:
                    nc.gpsimd.dma_start(out=L["t_lo"], in_=tlo[b])
                passes = [(s_hi, L["t_hi"]), (s_hi, L["t_lo"]),
                          (s_lo, L["t_hi"])]
            else:
                s_f = srcp.tile([128, 8, 128], f32, tag="sT", name="sT")
                nc.gpsimd.dma_start(out=s_f, in_=sT[b, k])
                passes = [(s_f, L["t_f"])]
            np_ = len(passes)
            for pi, (lh, rh) in enumerate(passes):
                for kc in range(8):
                    first = pi == 0 and kc == 0
                    last = pi == np_ - 1 and kc == 7
                    for th2 in range(2):
                        nc.tensor.matmul(
                            ps[:, th2 * 512:(th2 + 1) * 512],
                            lhsT=lh[:, kc, :],
                            rhs=rh[:, kc, th2 * 512:(th2 + 1) * 512],
                            start=first, stop=last)
            cache[key] = ps
            return ps

        for b in range(BPC):
            L = emit_loads(b)
            vsrcR_t, vsrcT_t = L["vsrcR"], L["vsrcT"]
            vsrc8_t, vscal_t, vsrcT2_t = L["vsrc8"], L["vscal"], L["vsrcT2"]
            addtB, addt_vec = L["addtB"], L["addt_vec"]

            negmall = accp.tile([128, NBLK], f32, tag="negmall", name="negmall")
            s1all = accp.tile([128, NBLK], f32, tag="s1all", name="s1all")
            s2all = accp.tile([128, NBLK], f32, tag="s2all", name="s2all")
            negx1 = [big.tile([128, S], f32, tag=f"negx1_{k}", name=f"negx1_{k}")
                     for k in range(NBLK)]
            d2cs = [csp.tile([1, 512], f32, tag=f"csA{h}", name=f"csA{h}")
                    for h in range(2)]
            dpcs = [csp.tile([1, 512], f32, tag=f"dpA{h}", name=f"dpA{h}")
                    for h in range(2)]

            # ---------------- sweep 1 ----------------
            for k in range(NBLK):
                ps = emit_fwd(b, k)
                # negx1 = -(scores + add_tgt); negm = -rowmax
                nc.vector.scalar_tensor_tensor(
                    out=negx1[k], in0=ps, scalar=-1.0, in1=addtB,
                    op0=op.mult, op1=op.subtract)
                nc.vector.tensor_reduce(
                    out=negmall[:, k:k + 1], in_=negx1[k],
                    axis=mybir.AxisListType.X, op=op.min)
                # s1 = rowsum(exp(x - m))
                junk = sc.tile([128, S], f32, tag="scx", name="junk")
                nc.scalar.activation(
                    out=junk, in_=negx1[k], func=act.Exp, scale=-1.0,
                    bias=negmall[:, k:k + 1], accum_out=s1all[:, k:k + 1])
                # s2 = rowsum(exp(x/tempS))
                junk2 = sc.tile([128, S], f32, tag="scx", name="junk2")
                nc.scalar.activation(
                    out=junk2, in_=negx1[k], func=act.Exp,
                    scale=vscal_t[:, 0:1], accum_out=s2all[:, k:k + 1])
                # e2t = exp((x + add_src)/tempT) -> column sums d'
                e2t_k = sc.tile([128, S], f16, tag="e2t", name="e2t")
                nc.scalar.activation(
                    out=e2t_k, in_=negx1[k], func=act.Exp,
                    scale=vscal_t[:, 1:2], bias=vsrcT_t[:, k:k + 1])
                # T2e = exp((x + add_src)/8) -> column sums d2 (stabilizer)
                t2e = sc.tile([128, S], bf16, tag="scx", name="t2e")
                nc.scalar.activation(
                    out=t2e, in_=negx1[k], func=act.Exp,
                    scale=-0.125, bias=vsrc8_t[:, k:k + 1])
                for h in range(2):
                    nc.tensor.matmul(
                        d2cs[h], lhsT=onesb, rhs=t2e[:, h * 512:(h + 1) * 512],
                        start=(k == 0), stop=(k == NBLK - 1))
                for h in range(2):
                    nc.tensor.matmul(
                        dpcs[h], lhsT=onesh, rhs=e2t_k[:, h * 512:(h + 1) * 512],
                        start=(k == 0), stop=(k == NBLK - 1))

            # pre-emit the next batch's first fwd blocks into the PE stream
            # so the tensor engine has work while this batch's column phase
            # (DVE/ACT-bound) runs.
            if b + 1 < BPC:
                for k in range(PREFETCH):
                    emit_fwd(b + 1, k)

            # ---------------- stabilizer / scale vectors ----------------
            lnd2 = vecp.tile([1, S], f32, tag="lnd2", name="lnd2")
            for h in range(2):
                nc.scalar.activation(out=lnd2[:, h * 512:(h + 1) * 512],
                                     in_=d2cs[h], func=act.Ln,
                                     bias=cEPS12[0:1, :])
            cpv = lnd2
            nc.vector.scalar_tensor_tensor(
                out=cpv, in0=lnd2, scalar=8.0, in1=addt_vec,
                op0=op.mult, op1=op.add)
            nc.vector.tensor_scalar_max(out=cpv, in0=cpv, scalar1=-300.0)
            cB = bc.tile([128, S], f32, tag="cB", name="cB")
            cDr = drp.tile([1, S], f32, tag="cDr", name="cDr")
            nc.sync.dma_start(out=cDr, in_=cpv)
            nc.gpsimd.dma_start(
                out=cB, in_=bass.AP(tensor=cDr.tensor, offset=cDr.offset,
                                    ap=[[0, 128]] + list(cDr.ap[1:])))

            # lndB = 0.5*ln(d' + 1e-12), broadcast (tanh z-argument)
            lnw = vecp.tile([1, S], f32, tag="lnw", name="lnw")
            for h in range(2):
                nc.scalar.activation(out=lnw[:, h * 512:(h + 1) * 512],
                                     in_=dpcs[h], func=act.Ln,
                                     bias=cEPS12[0:1, :])
            nc.vector.tensor_scalar_mul(out=lnw, in0=lnw, scalar1=0.5)
            wB = bc.tile([128, S], f32, tag="wB", name="wB")
            wDr = drp.tile([1, S], f32, tag="wDr", name="wDr")
            nc.sync.dma_start(out=wDr, in_=lnw)
            nc.gpsimd.dma_start(
                out=wB, in_=bass.AP(tensor=wDr.tensor, offset=wDr.offset,
                                    ap=[[0, 128]] + list(wDr.ap[1:])))

            lns1all = accp.tile([128, NBLK], f32, tag="lns1all", name="lns1all")
            nc.scalar.activation(out=lns1all, in_=s1all, func=act.Ln)
            ap8 = accp.tile([128, NBLK], f32, tag="ap8", name="ap8")
            nc.vector.scalar_tensor_tensor(
                out=ap8, in0=negmall, scalar=-LNTAU, in1=lns1all,
                op0=op.add, op1=op.subtract)
            lns2all = accp.tile([128, NBLK], f32, tag="lns2all", name="lns2all")
            nc.scalar.activation(out=lns2all, in_=s2all, func=act.Ln)
            ai8 = accp.tile([128, NBLK], f32, tag="ai8", name="ai8")
            nc.vector.scalar_tensor_tensor(
                out=ai8, in0=lns2all, scalar=0.5, in1=vsrcT2_t,
                op0=op.mult, op1=op.add)
            neglns2 = accp.tile([128, NBLK], f32, tag="neglns2", name="neglns2")
            nc.vector.tensor_scalar_mul(out=neglns2, in0=lns2all, scalar1=-1.0)

            spcs = [csp.tile([1, 512], f32, tag=f"csA{h}", name=f"spA{h}")
                    for h in range(2)]

            # ------- sweep 2: s' column sums + align_prob -------
            for k in range(NBLK):
                up = sc2.tile([128, S], f32, tag="scu", name="scu")
                nc.vector.scalar_tensor_tensor(
                    out=up, in0=negx1[k], scalar=vsrcR_t[:, k:k + 1], in1=cB,
                    op0=op.subtract, op1=op.add)
                nc.scalar.activation(out=up, in_=up, func=act.Exp, scale=-1.0)
                for h in range(2):
                    nc.tensor.matmul(
                        spcs[h], lhsT=onesS,
                        rhs=up[:, h * 512:(h + 1) * 512],
                        start=(k == 0), stop=(k == NBLK - 1))
                # z/2 = x*(1/tempS - 1/tempT)/2 - 0.5 ln d' (+A' in tanh bias)
                zt = sc2.tile([128, S], f32, tag="zt", name="zt")
                nc.vector.scalar_tensor_tensor(
                    out=zt, in0=negx1[k], scalar=vscal_t[:, 2:3], in1=wB,
                    op0=op.mult, op1=op.subtract)
                th = sc2.tile([128, S], f32, tag="th", name="th")
                nc.scalar.activation(out=th, in_=zt, func=act.Tanh,
                                     bias=ai8[:, k:k + 1])
                # p = exp(x/tempS - ln s2);  H = (1 + tanh)*p = 2pq/(p+q)
                pt = sc2.tile([128, S], f32, tag="pt", name="pt")
                nc.scalar.activation(
                    out=pt, in_=negx1[k], func=act.Exp,
                    scale=vscal_t[:, 0:1], bias=neglns2[:, k:k + 1])
                num = sc2.tile([128, S], f32, tag="num", name="num")
                nc.vector.scalar_tensor_tensor(
                    out=num, in0=th, scalar=1.0, in1=pt,
                    op0=op.add, op1=op.mult)
                nc.sync.dma_start(out=opr[b, k * 128:(k + 1) * 128, :], in_=num)

            # ---------------- threshold vector ----------------
            lnts = vecp.tile([1, S], f32, tag="lnts", name="lnts")
            for h in range(2):
                nc.scalar.activation(out=lnts[:, h * 512:(h + 1) * 512],
                                     in_=spcs[h], func=act.Ln,
                                     bias=cEPS38[0:1, :])
            thv = lnts
            nc.vector.scalar_tensor_tensor(
                out=thv, in0=lnts, scalar=-1.0, in1=cpv,
                op0=op.mult, op1=op.subtract)
            thB = bc.tile([128, S], f32, tag="thB", name="thB")
            thDr = drp.tile([1, S], f32, tag="thDr", name="thDr")
            nc.sync.dma_start(out=thDr, in_=thv)
            nc.gpsimd.dma_start(
                out=thB, in_=bass.AP(tensor=thDr.tensor, offset=thDr.offset,
                                     ap=[[0, 128]] + list(thDr.ap[1:])))

            # ---------------- sweep 3: alignment bits ----------------
            for k in range(NBLK):
                bit2 = sc2.tile([128, S], u8, tag="bit2", name="bit2")
                nc.vector.scalar_tensor_tensor(
                    out=bit2, in0=negx1[k], scalar=vsrcR_t[:, k:k + 1],
                    in1=thB, op0=op.subtract, op1=op.is_lt)
                algn = sc2.tile([128, S], u8, tag="algn", name="algn")
                nc.vector.scalar_tensor_tensor(
                    out=algn, in0=negx1[k], scalar=ap8[:, k:k + 1], in1=bit2,
                    op0=op.is_lt, op1=op.mult)
                nc.sync.dma_start(out=oal[b, k * 128:(k + 1) * 128, :],
                                  in_=algn)

    nc.finalize()
    return nc


def _get_nc():
    key = (MM_MODE, PREFETCH)
    if key not in _CACHE:
        _CACHE[key] = _build_bass()
    return _CACHE[key]


def _prep_host(h_src, h_tgt, ids_src, ids_tgt):
    f32 = np.float32
    m_src = ((ids_src == PAD_ID) | (ids_src == CLS_ID) | (ids_src == SEP_ID))
    m_tgt = ((ids_tgt == PAD_ID) | (ids_tgt == CLS_ID) | (ids_tgt == SEP_ID))
    add_src = np.where(m_src, f32(-10000.0), f32(0.0))
    add_tgt = np.where(m_tgt, f32(-10000.0), f32(0.0))
    len_src = (S - m_src.sum(1)).astype(f32)
    len_tgt = (S - m_tgt.sum(1)).astype(f32)
    tempS = np.sqrt(len_tgt)      # row-softmax temperature (p_src)
    tempT = np.sqrt(len_src)      # col-softmax temperature (p_tgt)

    hsT = np.ascontiguousarray(h_src.transpose(0, 2, 1)).astype(f32)
    htT = np.ascontiguousarray(h_tgt.transpose(0, 2, 1)).astype(f32)

    def lhs_layout(x):   # [B,D,S] -> [B, sblk, dpart, dchunk, s]
        y = x.reshape(B, 8, 128, NBLK, 128)
        return np.ascontiguousarray(y.transpose(0, 3, 2, 1, 4))

    def rhs_layout(x):   # [B,D,S] -> [B, dpart, dchunk, t]
        y = x.reshape(B, 8, 128, S)
        return np.ascontiguousarray(y.transpose(0, 2, 1, 3))

    inp = {}
    if MM_MODE == "f16x3":
        s_hi = hsT.astype(np.float16)
        s_lo = (hsT - s_hi.astype(f32)).astype(np.float16)
        t_hi = htT.astype(np.float16)
        t_lo = (htT - t_hi.astype(f32)).astype(np.float16)
        inp["shi"] = lhs_layout(s_hi)
        inp["slo"] = lhs_layout(s_lo)
        inp["thi"] = rhs_layout(t_hi)
        inp["tlo"] = rhs_layout(t_lo)
    else:
        inp["sT"] = lhs_layout(hsT)
        inp["tT"] = rhs_layout(htT)

    def col_layout(v):   # [B,S] -> [B, 128, NBLK]
        return np.ascontiguousarray(v.reshape(B, NBLK, 128).transpose(0, 2, 1))

    inp["vaddt"] = add_tgt
    inp["vsrcR"] = col_layout(add_src)
    inp["vsrcT"] = col_layout((add_src / tempT[:, None]).astype(f32))
    inp["vsrc8"] = col_layout((add_src * f32(0.125)).astype(f32))
    inp["vsrcT2"] = col_layout((add_src / (2.0 * tempT[:, None])).astype(f32))
    scal = np.zeros((B, 128, 3), f32)
    scal[:, :, 0] = (-1.0 / tempS)[:, None]
    scal[:, :, 1] = (-1.0 / tempT)[:, None]
    scal[:, :, 2] = ((1.0 / tempS - 1.0 / tempT) * 0.5)[:, None]
    inp["vscal"] = scal
    return inp


def kernel(hidden_states_src, hidden_states_tgt, inputs_src, inputs_tgt,
           _want_profile=False):
    from concourse.bass_utils import run_bass_kernel_spmd

    h_src = np.asarray(hidden_states_src, dtype=np.float32)
    h_tgt = np.asarray(hidden_states_tgt, dtype=np.float32)
    ids_src = np.asarray(inputs_src)
    ids_tgt = np.asarray(inputs_tgt)

    inp = _prep_host(h_src, h_tgt, ids_src, ids_tgt)
    nc = _get_nc()

    in_maps = []
    for c in range(NCORES):
        sl = slice(c * BPC, (c + 1) * BPC)
        in_maps.append({k: np.ascontiguousarray(v[sl]) for k, v in inp.items()})

    res = run_bass_kernel_spmd(nc, in_maps, list(range(NCORES)),
                               trace=_want_profile)

    align = np.empty((B, S, S), dtype=bool)
    prob = np.empty((B, S, S), dtype=np.float32)
    for c in range(NCORES):
        r = res.results[c]
        align[c * BPC:(c + 1) * BPC] = r["oal"].astype(bool)
        prob[c * BPC:(c + 1) * BPC] = r["opr"]
    out = (align[:, None], prob[:, None])
    if _want_profile:
        return out, res
    return out


# revision 24
# speedup vs baseline: 1.0078x; 1.0078x over previous
"""Trainium2 Bass kernel for nn_Aligner (sparse_attention).

Per batch b (B=16, S=1024, D=1024):
    scores   = h_src[b] @ h_tgt[b].T
    scores_s = scores + add_tgt[None,:]   # tgt-special-token cols masked
    scores_t = scores + add_src[:,None]   # src-special-token rows masked
    align      = (softmax_row(scores_s) > 1e-3) & (softmax_col(scores_t) > 1e-3)
    p          = softmax_row(scores_s / sqrt(len_tgt))
    q          = softmax_col(scores_t / sqrt(len_src))
    align_prob = 2 p q / (p + q + 1e-9)
Returns (align[:,None] bool, align_prob[:,None] float32).

Distribution: pure data parallel, 2 batches per core on 8 NeuronCores.

Device algorithm (all work stays in [s,t] layout; no [S,S] transposes):
  sweep1  PE: scores via fp16x3 split-product matmuls (hi/lo half
          decomposition of the fp32 operands: hi*hi + hi*lo + lo*hi, each
          product exact in the fp32 PSUM accumulate; max error ~1.4e-5,
          validated to flip zero threshold booleans). DVE folds in the
          tgt mask and takes the row max; ACT exp passes (with fused
          row-sum accumulators) produce row sums s1 (untempered) and s2
          (tempered), plus tempered column-sum operands whose column sums
          d2, d' are taken by PE ones-matmuls.
  The untempered column softmax is stabilized per column by a tempered
  log-sum-exp surrogate colmax c'_j = 8 ln(sum_i exp(x/8)) which lies in
  [colmax_j, colmax_j + 55.4] (clamped at -300 for fully-masked cols).
  sweep2  s'_j = sum_i exp(x + add_src - c') via DVE subtract + ACT exp +
          fp32 PE ones-matmul (fp32 because s' feeds a log-domain
          threshold with ~4e-5 budget; reduced-precision sums are not
          safe). align_prob is also produced here:
          2pq/(p+q) = p*(1+tanh(z/2)) with z = ln q - ln p an affine
          function of the scores, so the whole harmonic mean costs one
          DVE op, one ACT tanh, one ACT exp, one DVE combine per block
          (tanh lives in the same ACT table set as exp - no table-set
          reload thrash).
  sweep3  both alignment tests are done in log space:
            bit1:  x > m_i + ln(tau) + ln(s1_i)        (per-row scalar)
            bit2:  x + add_src > ln(tau s'_j) + c'_j   (per-col vector)
          fused into two DVE scalar_tensor_tensor ops producing uint8.
"""

import numpy as np

B, S, D = 16, 1024, 1024
NCORES = 8
BPC = B // NCORES
NBLK = S // 128
PAD_ID, CLS_ID, SEP_ID = 0, 101, 102
TAU = 1e-3
LNTAU = float(np.log(np.float32(TAU)))
MM_MODE = "f16x3"        # "f16x3" | "f32"
PREFETCH = 4             # next batch matmul blocks pre-emitted to the PE

_CACHE = {}


def _build_bass():
    from contextlib import ExitStack
    import concourse.bass as bass
    import concourse.tile as tile
    import concourse.mybir as mybir
    from concourse import bacc
    from concourse.mybir import AluOpType as op, ActivationFunctionType as act

    f32 = mybir.dt.float32
    f16 = mybir.dt.float16
    bf16 = mybir.dt.bfloat16
    u8 = mybir.dt.uint8

    nc = bacc.Bacc(None, target_bir_lowering=False)

    if MM_MODE == "f16x3":
        shi = nc.declare_dram_parameter("shi", [BPC, NBLK, 128, 8, 128], f16, isOutput=False)
        slo = nc.declare_dram_parameter("slo", [BPC, NBLK, 128, 8, 128], f16, isOutput=False)
        thi = nc.declare_dram_parameter("thi", [BPC, 128, 8, S], f16, isOutput=False)
        tlo = nc.declare_dram_parameter("tlo", [BPC, 128, 8, S], f16, isOutput=False)
    else:
        sT = nc.declare_dram_parameter("sT", [BPC, NBLK, 128, 8, 128], f32, isOutput=False)
        tT = nc.declare_dram_parameter("tT", [BPC, 128, 8, S], f32, isOutput=False)
    vaddt = nc.declare_dram_parameter("vaddt", [BPC, S], f32, isOutput=False)
    vsrcR = nc.declare_dram_parameter("vsrcR", [BPC, 128, NBLK], f32, isOutput=False)
    vsrcT = nc.declare_dram_parameter("vsrcT", [BPC, 128, NBLK], f32, isOutput=False)
    vsrc8 = nc.declare_dram_parameter("vsrc8", [BPC, 128, NBLK], f32, isOutput=False)
    vscal = nc.declare_dram_parameter("vscal", [BPC, 128, 3], f32, isOutput=False)
    vsrcT2 = nc.declare_dram_parameter("vsrcT2", [BPC, 128, NBLK], f32, isOutput=False)
    oal = nc.declare_dram_parameter("oal", [BPC, S, S], u8, isOutput=True)
    opr = nc.declare_dram_parameter("opr", [BPC, S, S], f32, isOutput=True)

    with tile.TileContext(nc) as tc, ExitStack() as es:
        io = es.enter_context(tc.tile_pool(name="io", bufs=1))
        srcp = es.enter_context(tc.tile_pool(name="srcp", bufs=4))
        big = es.enter_context(tc.tile_pool(name="big", bufs=1))
        sc = es.enter_context(tc.tile_pool(name="sc", bufs=4))
        sc2 = es.enter_context(tc.tile_pool(name="sc2", bufs=3))
        bc = es.enter_context(tc.tile_pool(name="bc", bufs=1))
        accp = es.enter_context(tc.tile_pool(name="accp", bufs=1))
        vecp = es.enter_context(tc.tile_pool(name="vecp", bufs=1))
        const = es.enter_context(tc.tile_pool(name="const", bufs=1))
        drp = es.enter_context(tc.tile_pool(name="drp", bufs=2, space="DRAM"))
        mmp = es.enter_context(tc.tile_pool(name="mmp", bufs=2, space="PSUM"))
        csp = es.enter_context(tc.tile_pool(name="csp", bufs=1, space="PSUM"))

        onesb = const.tile([128, 1], bf16, tag="onesb", name="onesb")
        nc.vector.memset(onesb, 1.0)
        onesh = const.tile([128, 1], f16, tag="onesh", name="onesh")
        nc.vector.memset(onesh, 1.0)
        onesS = const.tile([128, 1], f32, tag="onesS", name="onesS")
        nc.vector.memset(onesS, TAU)
        cEPS12 = const.tile([128, 1], f32, tag="cEPS12", name="cEPS12")
        nc.vector.memset(cEPS12, 1e-12)
        cEPS38 = const.tile([128, 1], f32, tag="cEPS38", name="cEPS38")
        nc.vector.memset(cEPS38, 1e-38)

        cache = {}

        def emit_loads(b):
            key = ("L", b)
            if key in cache:
                return cache[key]
            L = {}
            if MM_MODE == "f16x3":
                L["t_hi"] = io.tile([128, 8, S], f16, tag="thi", name="thi")
                L["t_lo"] = io.tile([128, 8, S], f16, tag="tlo", name="tlo")
                nc.gpsimd.dma_start(out=L["t_hi"][:, 0:2, :], in_=thi[b, :, 0:2, :])
                nc.gpsimd.dma_start(out=L["t_hi"][:, 2:8, :], in_=thi[b, :, 2:8, :])
            else:
                L["t_f"] = io.tile([128, 8, S], f32, tag="tT", name="tT")
                nc.gpsimd.dma_start(out=L["t_f"], in_=tT[b])
            for nm, src_t, w in (("vsrcR", vsrcR, NBLK), ("vsrcT", vsrcT, NBLK),
                                 ("vsrc8", vsrc8, NBLK), ("vscal", vscal, 3),
                                 ("vsrcT2", vsrcT2, NBLK)):
                L[nm] = io.tile([128, w], f32, tag=nm, name=nm)
                nc.gpsimd.dma_start(out=L[nm], in_=src_t[b])
            L["addt_vec"] = vecp.tile([1, S], f32, tag="addt_vec", name="addt_vec")
            nc.gpsimd.dma_start(out=L["addt_vec"], in_=vaddt[b:b + 1, :])
            L["addtB"] = bc.tile([128, S], f32, tag="addtB", name="addtB")
            vb = vaddt[b]
            nc.gpsimd.dma_start(
                out=L["addtB"],
                in_=bass.AP(tensor=vb.tensor, offset=vb.offset,
                            ap=[[0, 128]] + list(vb.ap)))
            cache[key] = L
            return L

        def emit_fwd(b, k):
            """Scores matmuls for one 128-row block -> PSUM tile."""
            key = ("F", b, k)
            if key in cache:
                return cache[key]
            L = emit_loads(b)
            ps = mmp.tile([128, S], f32, tag="mm", name="mm")
            if MM_MODE == "f16x3":
                s_hi = srcp.tile([128, 8, 128], f16, tag="shi", name="shi")
                s_lo = srcp.tile([128, 8, 128], f16, tag="slo", name="slo")
                nc.gpsimd.dma_start(out=s_hi, in_=shi[b, k])
                nc.gpsimd.dma_start(out=s_lo, in_=slo[b, k])
                if k == 0:
                    nc.gpsimd.dma_start(out=L["t_lo"][:, 0:2, :], in_=tlo[b, :, 0:2, :])
                    nc.gpsimd.dma_start(out=L["t_lo"][:, 2:8, :], in_=tlo[b, :, 2:8, :])
                passes = [(s_hi, L["t_hi"]), (s_hi, L["t_lo"]),
                          (s_lo, L["t_hi"])]
            else:
                s_f = srcp.tile([128, 8, 128], f32, tag="sT", name="sT")
                nc.gpsimd.dma_start(out=s_f, in_=sT[b, k])
                passes = [(s_f, L["t_f"])]
            np_ = len(passes)
            for pi, (lh, rh) in enumerate(passes):
                for kc in range(8):
                    first = pi == 0 and kc == 0
                    last = pi == np_ - 1 and kc == 7
                    for th2 in range(2):
                        nc.tensor.matmul(
                            ps[:, th2 * 512:(th2 + 1) * 512],
                            lhsT=lh[:, kc, :],
                            rhs=rh[:, kc, th2 * 512:(th2 + 1) * 512],
                            start=first, stop=last)
            cache[key] = ps
            return ps

        for b in range(BPC):
            L = emit_loads(b)
            vsrcR_t, vsrcT_t = L["vsrcR"], L["vsrcT"]
            vsrc8_t, vscal_t, vsrcT2_t = L["vsrc8"], L["vscal"], L["vsrcT2"]
            addtB, addt_vec = L["addtB"], L["addt_vec"]

            negmall = accp.tile([128, NBLK], f32, tag="negmall", name="negmall")
            s1all = accp.tile([128, NBLK], f32, tag="s1all", name="s1all")
            s2all = accp.tile([128, NBLK], f32, tag="s2all", name="s2all")
            negx1 = [big.tile([128, S], f32, tag=f"negx1_{k}", name=f"negx1_{k}")
                     for k in range(NBLK)]
            d2cs = [csp.tile([1, 512], f32, tag=f"csA{h}", name=f"csA{h}")
                    for h in range(2)]
            dpcs = [csp.tile([1, 512], f32, tag=f"dpA{h}", name=f"dpA{h}")
                    for h in range(2)]

            # ---------------- sweep 1 ----------------
            for k in range(NBLK):
                ps = emit_fwd(b, k)
                # negx1 = -(scores + add_tgt); negm = -rowmax
                nc.vector.scalar_tensor_tensor(
                    out=negx1[k], in0=ps, scalar=-1.0, in1=addtB,
                    op0=op.mult, op1=op.subtract)
                nc.vector.tensor_reduce(
                    out=negmall[:, k:k + 1], in_=negx1[k],
                    axis=mybir.AxisListType.X, op=op.min)
                # s1 = rowsum(exp(x - m))
                junk = sc.tile([128, S], f32, tag="scx", name="junk")
                nc.scalar.activation(
                    out=junk, in_=negx1[k], func=act.Exp, scale=-1.0,
                    bias=negmall[:, k:k + 1], accum_out=s1all[:, k:k + 1])
                # s2 = rowsum(exp(x/tempS))
                junk2 = sc.tile([128, S], f32, tag="scx", name="junk2")
                nc.scalar.activation(
                    out=junk2, in_=negx1[k], func=act.Exp,
                    scale=vscal_t[:, 0:1], accum_out=s2all[:, k:k + 1])
                # e2t = exp((x + add_src)/tempT) -> column sums d'
                e2t_k = sc.tile([128, S], f16, tag="e2t", name="e2t")
                nc.scalar.activation(
                    out=e2t_k, in_=negx1[k], func=act.Exp,
                    scale=vscal_t[:, 1:2], bias=vsrcT_t[:, k:k + 1])
                # T2e = exp((x + add_src)/8) -> column sums d2 (stabilizer)
                t2e = sc.tile([128, S], bf16, tag="scx", name="t2e")
                nc.scalar.activation(
                    out=t2e, in_=negx1[k], func=act.Exp,
                    scale=-0.125, bias=vsrc8_t[:, k:k + 1])
                for h in range(2):
                    nc.tensor.matmul(
                        d2cs[h], lhsT=onesb, rhs=t2e[:, h * 512:(h + 1) * 512],
                        start=(k == 0), stop=(k == NBLK - 1))
                for h in range(2):
                    nc.tensor.matmul(
                        dpcs[h], lhsT=onesh, rhs=e2t_k[:, h * 512:(h + 1) * 512],
                        start=(k == 0), stop=(k == NBLK - 1))

            # pre-emit the next batch's first fwd blocks into the PE stream
            # so the tensor engine has work while this batch's column phase
            # (DVE/ACT-bound) runs.
            if b + 1 < BPC:
                for k in range(PREFETCH):
                    emit_fwd(b + 1, k)

            # ---------------- stabilizer / scale vectors ----------------
            lnd2 = vecp.tile([1, S], f32, tag="lnd2", name="lnd2")
            for h in range(2):
                nc.scalar.activation(out=lnd2[:, h * 512:(h + 1) * 512],
                                     in_=d2cs[h], func=act.Ln,
                                     bias=cEPS12[0:1, :])
            cpv = lnd2
            nc.vector.scalar_tensor_tensor(
                out=cpv, in0=lnd2, scalar=8.0, in1=addt_vec,
                op0=op.mult, op1=op.add)
            nc.vector.tensor_scalar_max(out=cpv, in0=cpv, scalar1=-300.0)
            cB = bc.tile([128, S], f32, tag="cB", name="cB")
            cDr = drp.tile([1, S], f32, tag="cDr", name="cDr")
            nc.sync.dma_start(out=cDr, in_=cpv)
            nc.gpsimd.dma_start(
                out=cB, in_=bass.AP(tensor=cDr.tensor, offset=cDr.offset,
                                    ap=[[0, 128]] + list(cDr.ap[1:])))

            # lndB = 0.5*ln(d' + 1e-12), broadcast (tanh z-argument)
            lnw = vecp.tile([1, S], f32, tag="lnw", name="lnw")
            for h in range(2):
                nc.scalar.activation(out=lnw[:, h * 512:(h + 1) * 512],
                                     in_=dpcs[h], func=act.Ln,
                                     bias=cEPS12[0:1, :])
            nc.vector.tensor_scalar_mul(out=lnw, in0=lnw, scalar1=0.5)
            wB = bc.tile([128, S], f32, tag="wB", name="wB")
            wDr = drp.tile([1, S], f32, tag="wDr", name="wDr")
            nc.sync.dma_start(out=wDr, in_=lnw)
            nc.gpsimd.dma_start(
                out=wB, in_=bass.AP(tensor=wDr.tensor, offset=wDr.offset,
                                    ap=[[0, 128]] + list(wDr.ap[1:])))

            lns1all = accp.tile([128, NBLK], f32, tag="lns1all", name="lns1all")
            nc.scalar.activation(out=lns1all, in_=s1all, func=act.Ln)
            ap8 = accp.tile([128, NBLK], f32, tag="ap8", name="ap8")
            nc.vector.scalar_tensor_tensor(
                out=ap8, in0=negmall, scalar=-LNTAU, in1=lns1all,
                op0=op.add, op1=op.subtract)
            lns2all = accp.tile([128, NBLK], f32, tag="lns2all", name="lns2all")
            nc.scalar.activation(out=lns2all, in_=s2all, func=act.Ln)
            ai8 = accp.tile([128, NBLK], f32, tag="ai8", name="ai8")
            nc.vector.scalar_tensor_tensor(
                out=ai8, in0=lns2all, scalar=0.5, in1=vsrcT2_t,
                op0=op.mult, op1=op.add)
            neglns2 = accp.tile([128, NBLK], f32, tag="neglns2", name="neglns2")
            nc.vector.tensor_scalar_mul(out=neglns2, in0=lns2all, scalar1=-1.0)

            spcs = [csp.tile([1, 512], f32, tag=f"csA{h}", name=f"spA{h}")
                    for h in range(2)]

            # ------- sweep 2: s' column sums + align_prob -------
            for k in range(NBLK):
                up = sc2.tile([128, S], f32, tag="scu", name="scu")
                nc.vector.scalar_tensor_tensor(
                    out=up, in0=negx1[k], scalar=vsrcR_t[:, k:k + 1], in1=cB,
                    op0=op.subtract, op1=op.add)
                nc.scalar.activation(out=up, in_=up, func=act.Exp, scale=-1.0)
                for h in range(2):
                    nc.tensor.matmul(
                        spcs[h], lhsT=onesS,
                        rhs=up[:, h * 512:(h + 1) * 512],
                        start=(k == 0), stop=(k == NBLK - 1))
                # z/2 = x*(1/tempS - 1/tempT)/2 - 0.5 ln d' (+A' in tanh bias)
                zt = sc2.tile([128, S], f32, tag="zt", name="zt")
                nc.vector.scalar_tensor_tensor(
                    out=zt, in0=negx1[k], scalar=vscal_t[:, 2:3], in1=wB,
                    op0=op.mult, op1=op.subtract)
                th = sc2.tile([128, S], f32, tag="th", name="th")
                nc.scalar.activation(out=th, in_=zt, func=act.Tanh,
                                     bias=ai8[:, k:k + 1])
                # p = exp(x/tempS - ln s2);  H = (1 + tanh)*p = 2pq/(p+q)
                pt = sc2.tile([128, S], f32, tag="pt", name="pt")
                nc.scalar.activation(
                    out=pt, in_=negx1[k], func=act.Exp,
                    scale=vscal_t[:, 0:1], bias=neglns2[:, k:k + 1])
                num = sc2.tile([128, S], f32, tag="num", name="num")
                nc.vector.scalar_tensor_tensor(
                    out=num, in0=th, scalar=1.0, in1=pt,
                    op0=op.add, op1=op.mult)
                nc.sync.dma_start(out=opr[b, k * 128:(k + 1) * 128, :], in_=num)

            # ---------------- threshold vector ----------------
            lnts = vecp.tile([1, S], f32, tag="lnts", name="lnts")
            for h in range(2):
                nc.scalar.activation(out=lnts[:, h * 512:(h + 1) * 512],
                                     in_=spcs[h], func=act.Ln,
                                     bias=cEPS38[0:1, :])
            thv = lnts
            nc.vector.scalar_tensor_tensor(
                out=thv, in0=lnts, scalar=-1.0, in1=cpv,
                op0=op.mult, op1=op.subtract)
            thB = bc.tile([128, S], f32, tag="thB", name="thB")
            thDr = drp.tile([1, S], f32, tag="thDr", name="thDr")
            nc.sync.dma_start(out=thDr, in_=thv)
            nc.gpsimd.dma_start(
                out=thB, in_=bass.AP(tensor=thDr.tensor, offset=thDr.offset,
                                     ap=[[0, 128]] + list(thDr.ap[1:])))

            # ---------------- sweep 3: alignment bits ----------------
            for k in range(NBLK):
                bit2 = sc2.tile([128, S], u8, tag="bit2", name="bit2")
                nc.vector.scalar_tensor_tensor(
                    out=bit2, in0=negx1[k], scalar=vsrcR_t[:, k:k + 1],
                    in1=thB, op0=op.subtract, op1=op.is_lt)
                algn = sc2.tile([128, S], u8, tag="algn", name="algn")
                nc.vector.scalar_tensor_tensor(
                    out=algn, in0=negx1[k], scalar=ap8[:, k:k + 1], in1=bit2,
                    op0=op.is_lt, op1=op.mult)
                nc.sync.dma_start(out=oal[b, k * 128:(k + 1) * 128, :],
                                  in_=algn)

    nc.finalize()
    return nc


def _get_nc():
    key = (MM_MODE, PREFETCH)
    if key not in _CACHE:
        _CACHE[key] = _build_bass()
    return _CACHE[key]


def _prep_host(h_src, h_tgt, ids_src, ids_tgt):
    f32 = np.float32
    m_src = ((ids_src == PAD_ID) | (ids_src == CLS_ID) | (ids_src == SEP_ID))
    m_tgt = ((ids_tgt == PAD_ID) | (ids_tgt == CLS_ID) | (ids_tgt == SEP_ID))
    add_src = np.where(m_src, f32(-10000.0), f32(0.0))
    add_tgt = np.where(m_tgt, f32(-10000.0), f32(0.0))
    len_src = (S - m_src.sum(1)).astype(f32)
    len_tgt = (S - m_tgt.sum(1)).astype(f32)
    tempS = np.sqrt(len_tgt)      # row-softmax temperature (p_src)
    tempT = np.sqrt(len_src)      # col-softmax temperature (p_tgt)

    hsT = np.ascontiguousarray(h_src.transpose(0, 2, 1)).astype(f32)
    htT = np.ascontiguousarray(h_tgt.transpose(0, 2, 1)).astype(f32)

    def lhs_layout(x):   # [B,D,S] -> [B, sblk, dpart, dchunk, s]
        y = x.reshape(B, 8, 128, NBLK, 128)
        return np.ascontiguousarray(y.transpose(0, 3, 2, 1, 4))

    def rhs_layout(x):   # [B,D,S] -> [B, dpart, dchunk, t]
        y = x.reshape(B, 8, 128, S)
        return np.ascontiguousarray(y.transpose(0, 2, 1, 3))

    inp = {}
    if MM_MODE == "f16x3":
        s_hi = hsT.astype(np.float16)
        s_lo = (hsT - s_hi.astype(f32)).astype(np.float16)
        t_hi = htT.astype(np.float16)
        t_lo = (htT - t_hi.astype(f32)).astype(np.float16)
        inp["shi"] = lhs_layout(s_hi)
        inp["slo"] = lhs_layout(s_lo)
        inp["thi"] = rhs_layout(t_hi)
        inp["tlo"] = rhs_layout(t_lo)
    else:
        inp["sT"] = lhs_layout(hsT)
        inp["tT"] = rhs_layout(htT)

    def col_layout(v):   # [B,S] -> [B, 128, NBLK]
        return np.ascontiguousarray(v.reshape(B, NBLK, 128).transpose(0, 2, 1))

    inp["vaddt"] = add_tgt
    inp["vsrcR"] = col_layout(add_src)
    inp["vsrcT"] = col_layout((add_src / tempT[:, None]).astype(f32))
    inp["vsrc8"] = col_layout((add_src * f32(0.125)).astype(f32))
    inp["vsrcT2"] = col_layout((add_src / (2.0 * tempT[:, None])).astype(f32))
    scal = np.zeros((B, 128, 3), f32)
    scal[:, :, 0] = (-1.0 / tempS)[:, None]
    scal[:, :, 1] = (-1.0 / tempT)[:, None]
    scal[:, :, 2] = ((1.0 / tempS - 1.0 / tempT) * 0.5)[:, None]
    inp["vscal"] = scal
    return inp


def kernel(hidden_states_src, hidden_states_tgt, inputs_src, inputs_tgt,
           _want_profile=False):
    from concourse.bass_utils import run_bass_kernel_spmd

    h_src = np.asarray(hidden_states_src, dtype=np.float32)
    h_tgt = np.asarray(hidden_states_tgt, dtype=np.float32)
    ids_src = np.asarray(inputs_src)
    ids_tgt = np.asarray(inputs_tgt)

    inp = _prep_host(h_src, h_tgt, ids_src, ids_tgt)
    nc = _get_nc()

    in_maps = []
    for c in range(NCORES):
        sl = slice(c * BPC, (c + 1) * BPC)
        in_maps.append({k: np.ascontiguousarray(v[sl]) for k, v in inp.items()})

    res = run_bass_kernel_spmd(nc, in_maps, list(range(NCORES)),
                               trace=_want_profile)

    align = np.empty((B, S, S), dtype=bool)
    prob = np.empty((B, S, S), dtype=np.float32)
    for c in range(NCORES):
        r = res.results[c]
        align[c * BPC:(c + 1) * BPC] = r["oal"].astype(bool)
        prob[c * BPC:(c + 1) * BPC] = r["opr"]
    out = (align[:, None], prob[:, None])
    if _want_profile:
        return out, res
    return out


# revision 27
# speedup vs baseline: 1.0678x; 1.0596x over previous
"""Trainium2 Bass kernel for nn_Aligner (sparse_attention).

Per batch b (B=16, S=1024, D=1024):
    scores   = h_src[b] @ h_tgt[b].T
    scores_s = scores + add_tgt[None,:]   # tgt-special-token cols masked
    scores_t = scores + add_src[:,None]   # src-special-token rows masked
    align      = (softmax_row(scores_s) > 1e-3) & (softmax_col(scores_t) > 1e-3)
    p          = softmax_row(scores_s / sqrt(len_tgt))
    q          = softmax_col(scores_t / sqrt(len_src))
    align_prob = 2 p q / (p + q + 1e-9)
Returns (align[:,None] bool, align_prob[:,None] float32).

Distribution: pure data parallel, 2 batches per core on 8 NeuronCores.

Device algorithm (all work stays in [s,t] layout; no [S,S] transposes):
  sweep1  PE: scores via fp16x3 split-product matmuls (hi/lo half
          decomposition of the fp32 operands: hi*hi + hi*lo + lo*hi, each
          product exact in the fp32 PSUM accumulate; max error ~1.4e-5,
          validated to flip zero threshold booleans). DVE folds in the
          tgt mask and takes the row max; ACT exp passes (with fused
          row-sum accumulators) produce row sums s1 (untempered) and s2
          (tempered), plus tempered column-sum operands whose column sums
          d2, d' are taken by PE ones-matmuls.
  The untempered column softmax is stabilized per column by a tempered
  log-sum-exp surrogate colmax c'_j = 8 ln(sum_i exp(x/8)) which lies in
  [colmax_j, colmax_j + 55.4] (clamped at -300 for fully-masked cols).
  sweep2  s'_j = sum_i exp(x + add_src - c') via DVE subtract + ACT exp +
          fp32 PE ones-matmul (fp32 because s' feeds a log-domain
          threshold with ~4e-5 budget; reduced-precision sums are not
          safe). align_prob is also produced here:
          2pq/(p+q) = p*(1+tanh(z/2)) with z = ln q - ln p an affine
          function of the scores, so the whole harmonic mean costs one
          DVE op, one ACT tanh, one ACT exp, one DVE combine per block
          (tanh lives in the same ACT table set as exp - no table-set
          reload thrash).
  sweep3  both alignment tests are done in log space:
            bit1:  x > m_i + ln(tau) + ln(s1_i)        (per-row scalar)
            bit2:  x + add_src > ln(tau s'_j) + c'_j   (per-col vector)
          fused into two DVE scalar_tensor_tensor ops producing uint8.
"""

import numpy as np

B, S, D = 16, 1024, 1024
NCORES = 8
BPC = B // NCORES
NBLK = S // 128
PAD_ID, CLS_ID, SEP_ID = 0, 101, 102
TAU = 1e-3
LNTAU = float(np.log(np.float32(TAU)))
MM_MODE = "f16x3"        # "f16x3" | "f32"
PREFETCH = 4             # next batch matmul blocks pre-emitted to the PE

_CACHE = {}


def _build_bass():
    from contextlib import ExitStack
    import concourse.bass as bass
    import concourse.tile as tile
    import concourse.mybir as mybir
    from concourse import bacc
    from concourse.mybir import AluOpType as op, ActivationFunctionType as act

    f32 = mybir.dt.float32
    f16 = mybir.dt.float16
    bf16 = mybir.dt.bfloat16
    u8 = mybir.dt.uint8

    nc = bacc.Bacc(None, target_bir_lowering=False)

    if MM_MODE == "f16x3":
        shi = nc.declare_dram_parameter("shi", [BPC, NBLK, 128, 8, 128], f16, isOutput=False)
        slo = nc.declare_dram_parameter("slo", [BPC, NBLK, 128, 8, 128], f16, isOutput=False)
        thi = nc.declare_dram_parameter("thi", [BPC, 128, 8, S], f16, isOutput=False)
        tlo = nc.declare_dram_parameter("tlo", [BPC, 128, 8, S], f16, isOutput=False)
    else:
        sT = nc.declare_dram_parameter("sT", [BPC, NBLK, 128, 8, 128], f32, isOutput=False)
        tT = nc.declare_dram_parameter("tT", [BPC, 128, 8, S], f32, isOutput=False)
    vaddt = nc.declare_dram_parameter("vaddt", [BPC, S], f32, isOutput=False)
    vsrcR = nc.declare_dram_parameter("vsrcR", [BPC, 128, NBLK], f32, isOutput=False)
    vsrcT = nc.declare_dram_parameter("vsrcT", [BPC, 128, NBLK], f32, isOutput=False)
    vsrc8 = nc.declare_dram_parameter("vsrc8", [BPC, 128, NBLK], f32, isOutput=False)
    vscal = nc.declare_dram_parameter("vscal", [BPC, 128, 3], f32, isOutput=False)
    vsrcT2 = nc.declare_dram_parameter("vsrcT2", [BPC, 128, NBLK], f32, isOutput=False)
    oal = nc.declare_dram_parameter("oal", [BPC, S, S], u8, isOutput=True)
    opr = nc.declare_dram_parameter("opr", [BPC, S, S], f32, isOutput=True)

    with tile.TileContext(nc) as tc, ExitStack() as es:
        io = es.enter_context(tc.tile_pool(name="io", bufs=1))
        srcp = es.enter_context(tc.tile_pool(name="srcp", bufs=3))
        big = es.enter_context(tc.tile_pool(name="big", bufs=1))
        sc = es.enter_context(tc.tile_pool(name="sc", bufs=2))
        sc2 = es.enter_context(tc.tile_pool(name="sc2", bufs=2))
        bc = es.enter_context(tc.tile_pool(name="bc", bufs=1))
        accp = es.enter_context(tc.tile_pool(name="accp", bufs=1))
        vecp = es.enter_context(tc.tile_pool(name="vecp", bufs=1))
        const = es.enter_context(tc.tile_pool(name="const", bufs=1))
        drp = es.enter_context(tc.tile_pool(name="drp", bufs=2, space="DRAM"))
        mmp = es.enter_context(tc.tile_pool(name="mmp", bufs=2, space="PSUM"))
        csp = es.enter_context(tc.tile_pool(name="csp", bufs=1, space="PSUM"))

        onesb = const.tile([128, 1], bf16, tag="onesb", name="onesb")
        nc.vector.memset(onesb, 1.0)
        onesh = const.tile([128, 1], f16, tag="onesh", name="onesh")
        nc.vector.memset(onesh, 1.0)
        onesS = const.tile([128, 1], f32, tag="onesS", name="onesS")
        nc.vector.memset(onesS, TAU)
        cEPS12 = const.tile([128, 1], f32, tag="cEPS12", name="cEPS12")
        nc.vector.memset(cEPS12, 1e-12)
        cEPS38 = const.tile([128, 1], f32, tag="cEPS38", name="cEPS38")
        nc.vector.memset(cEPS38, 1e-38)

        cache = {}

        def emit_loads(b):
            key = ("L", b)
            if key in cache:
                return cache[key]
            L = {}
            if MM_MODE == "f16x3":
                L["t_hi"] = io.tile([128, 8, S], f16, tag="thi", name="thi")
                L["t_lo"] = io.tile([128, 8, S], f16, tag="tlo", name="tlo")
                nc.gpsimd.dma_start(out=L["t_hi"][:, 0:2, :], in_=thi[b, :, 0:2, :])
                nc.gpsimd.dma_start(out=L["t_hi"][:, 2:8, :], in_=thi[b, :, 2:8, :])
            else:
                L["t_f"] = io.tile([128, 8, S], f32, tag="tT", name="tT")
                nc.gpsimd.dma_start(out=L["t_f"], in_=tT[b])
            for nm, src_t, w in (("vsrcR", vsrcR, NBLK), ("vsrcT", vsrcT, NBLK),
                                 ("vsrc8", vsrc8, NBLK), ("vscal", vscal, 3),
                                 ("vsrcT2", vsrcT2, NBLK)):
                L[nm] = io.tile([128, w], f32, tag=nm, name=nm)
                nc.gpsimd.dma_start(out=L[nm], in_=src_t[b])
            L["addt_vec"] = vecp.tile([1, S], f32, tag="addt_vec", name="addt_vec")
            nc.gpsimd.dma_start(out=L["addt_vec"], in_=vaddt[b:b + 1, :])
            L["addtB"] = bc.tile([128, S], f32, tag="addtB", name="addtB")
            vb = vaddt[b]
            nc.gpsimd.dma_start(
                out=L["addtB"],
                in_=bass.AP(tensor=vb.tensor, offset=vb.offset,
                            ap=[[0, 128]] + list(vb.ap)))
            cache[key] = L
            return L

        def emit_fwd(b, k):
            """Scores matmuls for one 128-row block -> PSUM tile."""
            key = ("F", b, k)
            if key in cache:
                return cache[key]
            L = emit_loads(b)
            ps = mmp.tile([128, S], f32, tag="mm", name="mm")
            if MM_MODE == "f16x3":
                s_hi = srcp.tile([128, 8, 128], f16, tag="shi", name="shi")
                s_lo = srcp.tile([128, 8, 128], f16, tag="slo", name="slo")
                nc.gpsimd.dma_start(out=s_hi, in_=shi[b, k])
                nc.gpsimd.dma_start(out=s_lo, in_=slo[b, k])
                if k == 0:
                    nc.gpsimd.dma_start(out=L["t_lo"][:, 0:2, :], in_=tlo[b, :, 0:2, :])
                    nc.gpsimd.dma_start(out=L["t_lo"][:, 2:8, :], in_=tlo[b, :, 2:8, :])
                passes = [(s_hi, L["t_hi"]), (s_hi, L["t_lo"]),
                          (s_lo, L["t_hi"])]
            else:
                s_f = srcp.tile([128, 8, 128], f32, tag="sT", name="sT")
                nc.gpsimd.dma_start(out=s_f, in_=sT[b, k])
                passes = [(s_f, L["t_f"])]
            np_ = len(passes)
            for pi, (lh, rh) in enumerate(passes):
                for kc in range(8):
                    first = pi == 0 and kc == 0
                    last = pi == np_ - 1 and kc == 7
                    for th2 in range(2):
                        nc.tensor.matmul(
                            ps[:, th2 * 512:(th2 + 1) * 512],
                            lhsT=lh[:, kc, :],
                            rhs=rh[:, kc, th2 * 512:(th2 + 1) * 512],
                            start=first, stop=last)
            cache[key] = ps
            return ps

        def emit_batch_state(b):
            key = ("S", b)
            if key in cache:
                return cache[key]
            st = {"L": emit_loads(b)}
            st["negmall"] = accp.tile([128, NBLK], f32, tag="negmall", name="negmall")
            st["s1all"] = accp.tile([128, NBLK], f32, tag="s1all", name="s1all")
            st["s2all"] = accp.tile([128, NBLK], f32, tag="s2all", name="s2all")
            st["negx1"] = [big.tile([128, S], f32, tag=f"negx1_{k}",
                                    name=f"negx1_{k}", bufs=(2 if k < 2 else 1))
                           for k in range(NBLK)]
            cache[key] = st
            return st

        def emit_sweep1_block(b, k):
            """DVE/ACT sweep-1 pipeline for one block (colsum mms excluded).
            Returns (e2t, t2e) tiles for the deferred colsum matmuls."""
            key = ("B", b, k)
            if key in cache:
                return cache[key]
            st = emit_batch_state(b)
            L = st["L"]
            ps = emit_fwd(b, k)
            negx1 = st["negx1"]
            nc.vector.scalar_tensor_tensor(
                out=negx1[k], in0=ps, scalar=-1.0, in1=L["addtB"],
                op0=op.mult, op1=op.subtract)
            nc.vector.tensor_reduce(
                out=st["negmall"][:, k:k + 1], in_=negx1[k],
                axis=mybir.AxisListType.X, op=op.min)
            junk = sc.tile([128, S], f32, tag="scx", name="junk")
            nc.scalar.activation(
                out=junk, in_=negx1[k], func=act.Exp, scale=-1.0,
                bias=st["negmall"][:, k:k + 1],
                accum_out=st["s1all"][:, k:k + 1])
            junk2 = sc.tile([128, S], f32, tag="scx", name="junk2")
            nc.scalar.activation(
                out=junk2, in_=negx1[k], func=act.Exp,
                scale=L["vscal"][:, 0:1], accum_out=st["s2all"][:, k:k + 1])
            e2t_k = sc.tile([128, S], f16, tag=f"e2t{k % 3}", name="e2t")
            nc.scalar.activation(
                out=e2t_k, in_=negx1[k], func=act.Exp,
                scale=L["vscal"][:, 1:2], bias=L["vsrcT"][:, k:k + 1])
            t2e = sc.tile([128, S], bf16, tag=f"t2e{k % 3}", name="t2e")
            nc.scalar.activation(
                out=t2e, in_=negx1[k], func=act.Exp,
                scale=-0.125, bias=L["vsrc8"][:, k:k + 1])
            cache[key] = (e2t_k, t2e)
            return cache[key]

        for b in range(BPC):
            st = emit_batch_state(b)
            L = st["L"]
            vsrcR_t, vsrcT_t = L["vsrcR"], L["vsrcT"]
            vsrc8_t, vscal_t, vsrcT2_t = L["vsrc8"], L["vscal"], L["vsrcT2"]
            addtB, addt_vec = L["addtB"], L["addt_vec"]
            negmall, s1all, s2all = st["negmall"], st["s1all"], st["s2all"]
            negx1 = st["negx1"]

            d2cs = [csp.tile([1, 512], f32, tag=f"csA{h}", name=f"csA{h}")
                    for h in range(2)]
            dpcs = [csp.tile([1, 512], f32, tag=f"dpA{h}", name=f"dpA{h}")
                    for h in range(2)]

            # ---------------- sweep 1 ----------------
            for k in range(NBLK):
                e2t_k, t2e = emit_sweep1_block(b, k)
                for h in range(2):
                    nc.tensor.matmul(
                        d2cs[h], lhsT=onesb, rhs=t2e[:, h * 512:(h + 1) * 512],
                        start=(k == 0), stop=(k == NBLK - 1))
                for h in range(2):
                    nc.tensor.matmul(
                        dpcs[h], lhsT=onesh, rhs=e2t_k[:, h * 512:(h + 1) * 512],
                        start=(k == 0), stop=(k == NBLK - 1))

            # pre-emit the next batch's first blocks (matmuls AND the
            # DVE/ACT sweep-1 pipeline) so both PE and the PSUM mm slots
            # keep moving while this batch's column phase runs. The
            # colsum matmuls for those blocks are deferred to the next
            # batch's own sweep-1 loop (PSUM accumulator slots are still
            # owned by this batch here).
            if b + 1 < BPC:
                nb = emit_batch_state(b + 1)
                for k in range(PREFETCH):
                    emit_sweep1_block(b + 1, k)

            # ---------------- stabilizer / scale vectors ----------------
            lnd2 = vecp.tile([1, S], f32, tag="lnd2", name="lnd2")
            for h in range(2):
                nc.scalar.activation(out=lnd2[:, h * 512:(h + 1) * 512],
                                     in_=d2cs[h], func=act.Ln,
                                     bias=cEPS12[0:1, :])
            cpv = lnd2
            nc.vector.scalar_tensor_tensor(
                out=cpv, in0=lnd2, scalar=8.0, in1=addt_vec,
                op0=op.mult, op1=op.add)
            nc.vector.tensor_scalar_max(out=cpv, in0=cpv, scalar1=-300.0)
            cB = bc.tile([128, S], f32, tag="cB", name="cB")
            cDr = drp.tile([1, S], f32, tag="cDr", name="cDr")
            nc.sync.dma_start(out=cDr, in_=cpv)
            nc.gpsimd.dma_start(
                out=cB, in_=bass.AP(tensor=cDr.tensor, offset=cDr.offset,
                                    ap=[[0, 128]] + list(cDr.ap[1:])))

            # lndB = 0.5*ln(d' + 1e-12), broadcast (tanh z-argument)
            lnw = vecp.tile([1, S], f32, tag="lnw", name="lnw")
            for h in range(2):
                nc.scalar.activation(out=lnw[:, h * 512:(h + 1) * 512],
                                     in_=dpcs[h], func=act.Ln,
                                     bias=cEPS12[0:1, :])
            nc.vector.tensor_scalar_mul(out=lnw, in0=lnw, scalar1=0.5)
            wB = bc.tile([128, S], f32, tag="wB", name="wB")
            wDr = drp.tile([1, S], f32, tag="wDr", name="wDr")
            nc.sync.dma_start(out=wDr, in_=lnw)
            nc.gpsimd.dma_start(
                out=wB, in_=bass.AP(tensor=wDr.tensor, offset=wDr.offset,
                                    ap=[[0, 128]] + list(wDr.ap[1:])))

            lns1all = accp.tile([128, NBLK], f32, tag="lns1all", name="lns1all")
            nc.scalar.activation(out=lns1all, in_=s1all, func=act.Ln)
            ap8 = accp.tile([128, NBLK], f32, tag="ap8", name="ap8")
            nc.vector.scalar_tensor_tensor(
                out=ap8, in0=negmall, scalar=-LNTAU, in1=lns1all,
                op0=op.add, op1=op.subtract)
            lns2all = accp.tile([128, NBLK], f32, tag="lns2all", name="lns2all")
            nc.scalar.activation(out=lns2all, in_=s2all, func=act.Ln)
            ai8 = accp.tile([128, NBLK], f32, tag="ai8", name="ai8")
            nc.vector.scalar_tensor_tensor(
                out=ai8, in0=lns2all, scalar=0.5, in1=vsrcT2_t,
                op0=op.mult, op1=op.add)
            neglns2 = accp.tile([128, NBLK], f32, tag="neglns2", name="neglns2")
            nc.vector.tensor_scalar_mul(out=neglns2, in0=lns2all, scalar1=-1.0)

            spcs = [csp.tile([1, 512], f32, tag=f"csA{h}", name=f"spA{h}")
                    for h in range(2)]

            # ------- sweep 2: s' column sums + align_prob -------
            for k in range(NBLK):
                up = sc2.tile([128, S], f32, tag="scu", name="scu")
                nc.vector.scalar_tensor_tensor(
                    out=up, in0=negx1[k], scalar=vsrcR_t[:, k:k + 1], in1=cB,
                    op0=op.subtract, op1=op.add)
                nc.scalar.activation(out=up, in_=up, func=act.Exp, scale=-1.0)
                for h in range(2):
                    nc.tensor.matmul(
                        spcs[h], lhsT=onesS,
                        rhs=up[:, h * 512:(h + 1) * 512],
                        start=(k == 0), stop=(k == NBLK - 1))
                # z/2 = x*(1/tempS - 1/tempT)/2 - 0.5 ln d' (+A' in tanh bias)
                zt = sc2.tile([128, S], f32, tag="zt", name="zt")
                nc.vector.scalar_tensor_tensor(
                    out=zt, in0=negx1[k], scalar=vscal_t[:, 2:3], in1=wB,
                    op0=op.mult, op1=op.subtract)
                th = sc2.tile([128, S], f32, tag="th", name="th")
                nc.scalar.activation(out=th, in_=zt, func=act.Tanh,
                                     bias=ai8[:, k:k + 1])
                # p = exp(x/tempS - ln s2);  H = (1 + tanh)*p = 2pq/(p+q)
                pt = sc2.tile([128, S], f32, tag="pt", name="pt")
                nc.scalar.activation(
                    out=pt, in_=negx1[k], func=act.Exp,
                    scale=vscal_t[:, 0:1], bias=neglns2[:, k:k + 1])
                num = sc2.tile([128, S], f32, tag="num", name="num")
                nc.vector.scalar_tensor_tensor(
                    out=num, in0=th, scalar=1.0, in1=pt,
                    op0=op.add, op1=op.mult)
                nc.sync.dma_start(out=opr[b, k * 128:(k + 1) * 128, :], in_=num)

            # ---------------- threshold vector ----------------
            lnts = vecp.tile([1, S], f32, tag="lnts", name="lnts")
            for h in range(2):
                nc.scalar.activation(out=lnts[:, h * 512:(h + 1) * 512],
                                     in_=spcs[h], func=act.Ln,
                                     bias=cEPS38[0:1, :])
            thv = lnts
            nc.vector.scalar_tensor_tensor(
                out=thv, in0=lnts, scalar=-1.0, in1=cpv,
                op0=op.mult, op1=op.subtract)
            thB = bc.tile([128, S], f32, tag="thB", name="thB")
            thDr = drp.tile([1, S], f32, tag="thDr", name="thDr")
            nc.sync.dma_start(out=thDr, in_=thv)
            nc.gpsimd.dma_start(
                out=thB, in_=bass.AP(tensor=thDr.tensor, offset=thDr.offset,
                                     ap=[[0, 128]] + list(thDr.ap[1:])))

            # ---------------- sweep 3: alignment bits ----------------
            for k in range(NBLK):
                bit2 = sc2.tile([128, S], u8, tag="bit2", name="bit2")
                nc.vector.scalar_tensor_tensor(
                    out=bit2, in0=negx1[k], scalar=vsrcR_t[:, k:k + 1],
                    in1=thB, op0=op.subtract, op1=op.is_lt)
                algn = sc2.tile([128, S], u8, tag="algn", name="algn")
                nc.vector.scalar_tensor_tensor(
                    out=algn, in0=negx1[k], scalar=ap8[:, k:k + 1], in1=bit2,
                    op0=op.is_lt, op1=op.mult)
                nc.sync.dma_start(out=oal[b, k * 128:(k + 1) * 128, :],
                                  in_=algn)

    nc.finalize()
    return nc


def _get_nc():
    key = (MM_MODE, PREFETCH)
    if key not in _CACHE:
        _CACHE[key] = _build_bass()
    return _CACHE[key]


def _prep_host(h_src, h_tgt, ids_src, ids_tgt):
    f32 = np.float32
    m_src = ((ids_src == PAD_ID) | (ids_src == CLS_ID) | (ids_src == SEP_ID))
    m_tgt = ((ids_tgt == PAD_ID) | (ids_tgt == CLS_ID) | (ids_tgt == SEP_ID))
    add_src = np.where(m_src, f32(-10000.0), f32(0.0))
    add_tgt = np.where(m_tgt, f32(-10000.0), f32(0.0))
    len_src = (S - m_src.sum(1)).astype(f32)
    len_tgt = (S - m_tgt.sum(1)).astype(f32)
    tempS = np.sqrt(len_tgt)      # row-softmax temperature (p_src)
    tempT = np.sqrt(len_src)      # col-softmax temperature (p_tgt)

    hsT = np.ascontiguousarray(h_src.transpose(0, 2, 1)).astype(f32)
    htT = np.ascontiguousarray(h_tgt.transpose(0, 2, 1)).astype(f32)

    def lhs_layout(x):   # [B,D,S] -> [B, sblk, dpart, dchunk, s]
        y = x.reshape(B, 8, 128, NBLK, 128)
        return np.ascontiguousarray(y.transpose(0, 3, 2, 1, 4))

    def rhs_layout(x):   # [B,D,S] -> [B, dpart, dchunk, t]
        y = x.reshape(B, 8, 128, S)
        return np.ascontiguousarray(y.transpose(0, 2, 1, 3))

    inp = {}
    if MM_MODE == "f16x3":
        s_hi = hsT.astype(np.float16)
        s_lo = (hsT - s_hi.astype(f32)).astype(np.float16)
        t_hi = htT.astype(np.float16)
        t_lo = (htT - t_hi.astype(f32)).astype(np.float16)
        inp["shi"] = lhs_layout(s_hi)
        inp["slo"] = lhs_layout(s_lo)
        inp["thi"] = rhs_layout(t_hi)
        inp["tlo"] = rhs_layout(t_lo)
    else:
        inp["sT"] = lhs_layout(hsT)
        inp["tT"] = rhs_layout(htT)

    def col_layout(v):   # [B,S] -> [B, 128, NBLK]
        return np.ascontiguousarray(v.reshape(B, NBLK, 128).transpose(0, 2, 1))

    inp["vaddt"] = add_tgt
    inp["vsrcR"] = col_layout(add_src)
    inp["vsrcT"] = col_layout((add_src / tempT[:, None]).astype(f32))
    inp["vsrc8"] = col_layout((add_src * f32(0.125)).astype(f32))
    inp["vsrcT2"] = col_layout((add_src / (2.0 * tempT[:, None])).astype(f32))
    scal = np.zeros((B, 128, 3), f32)
    scal[:, :, 0] = (-1.0 / tempS)[:, None]
    scal[:, :, 1] = (-1.0 / tempT)[:, None]
    scal[:, :, 2] = ((1.0 / tempS - 1.0 / tempT) * 0.5)[:, None]
    inp["vscal"] = scal
    return inp


def kernel(hidden_states_src, hidden_states_tgt, inputs_src, inputs_tgt,
           _want_profile=False):
    from concourse.bass_utils import run_bass_kernel_spmd

    h_src = np.asarray(hidden_states_src, dtype=np.float32)
    h_tgt = np.asarray(hidden_states_tgt, dtype=np.float32)
    ids_src = np.asarray(inputs_src)
    ids_tgt = np.asarray(inputs_tgt)

    inp = _prep_host(h_src, h_tgt, ids_src, ids_tgt)
    nc = _get_nc()

    in_maps = []
    for c in range(NCORES):
        sl = slice(c * BPC, (c + 1) * BPC)
        in_maps.append({k: np.ascontiguousarray(v[sl]) for k, v in inp.items()})

    res = run_bass_kernel_spmd(nc, in_maps, list(range(NCORES)),
                               trace=_want_profile)

    align = np.empty((B, S, S), dtype=bool)
    prob = np.empty((B, S, S), dtype=np.float32)
    for c in range(NCORES):
        r = res.results[c]
        align[c * BPC:(c + 1) * BPC] = r["oal"].astype(bool)
        prob[c * BPC:(c + 1) * BPC] = r["opr"]
    out = (align[:, None], prob[:, None])
    if _want_profile:
        return out, res
    return out
